# revision 1
# baseline (speedup 1.0000x reference)
"""Trainium2 Bass kernel for cosine-similarity hint attention.

Computation (per batch b):
  sp = state_emb @ Ws.T + bs                  (B, A)
  hp = hints_emb @ Wh.T + bh                  (B, N, A)
  scores = <sp, hp> / (max(|sp|,eps) * max(|hp|,eps))
  attn = softmax(scores, axis=N)
  out = attn @ hints_emb                      (B, HD)

Distribution: data-parallel over batch, B=512 -> 64 per core on 8 cores.
Weights replicated. No collectives.

Device-side algorithm (per core, pipelined over 16 half-groups of 4
batches = 8 row-tiles of 128 hint-rows each):
  - hints arrive in two host-prepared fp8-e4m3 layouts: natural
    [r, h] tiles (for the final weighted sum, contraction over rows)
    and transposed [h, r] tiles (for the hint projection, contraction
    over h); the projection weights ship pre-scaled (S*Wh.T) in fp8 so
    no on-chip weight prep gates the first matmul. The natural copy is
    quantized with ERROR DIFFUSION along the hint-row axis (per batch):
    the weighted sum averages ~N rows with near-uniform softmax
    weights, so consecutive quantization errors telescope -- measured
    4.2e-3 total error (vs 2.7e-2 with round-to-nearest fp8 and
    3.4e-3 with bf16) at HALF the bf16 HBM bytes. Host-side work is
    layout/precision only (the quantizer reads only the hints
    tensor).
  - hint projection z' = X @ (S*Wh.T) runs on TensorE in fp8 DoubleRow
    mode (2 contraction k-tiles per pass, 0.5 cycles/row). A second,
    tiny DoubleRow matmul per tile with moving operand [S_q*q_b |
    S^2*wb] (q_b = sp_b @ Wh, wb = Wh.T @ bh) drops zq' = S_q*<q_b,x_r>
    and zw' = S^2*<x_r,wb> into a per-half [128, 16] PSUM tile, so no
    per-tile PSUM-extract copies are needed; one [128,16] SBUF mirror
    copy per half frees that PSUM bank early.
  - |hp_r|^2 (S^2-scaled) = sum(z'^2) + 2*zw' + S^2*(|bh|^2 + eps^2):
    ScalarE squares each z' tile (Square, no accumulator read) and
    VectorE reduces the squares; the constant rides a [128,1] bcast
    made with a stride-0-stationary PE matmul. 1/norm =
    exp(-0.5*ln(.)), with the S factor folded into rsn's prescale, so
    Square/Ln/Exp all live in one ACT table (no reloads).
  - softmax needs no max-subtraction (cosine scores are in [-1,1]);
    exp(scores) is written by ScalarE DIRECTLY into the block-diagonal
    bf16 stationary (strided activation output, no scatter copies);
    all 8 batches of a group accumulate in one [8, 512] PSUM tile over
    16 weighted-sum matmuls; the normalizer takes column-sums of the
    whole block-diagonal (ones-matmul + strided reduce) and divides at
    the output copy.
  - emission is software-pipelined: each chunk's PE-heavy tail
    (weighted sum + normalizer) is deferred ~4 chunks so TensorE never
    head-of-line blocks on the DVE/ACT epilogue chain; the deferral
    collapses to zero near the end, and the LAST group is processed in
    single-batch chunks so the un-overlappable drain (squares ->
    epilogue -> weighted sum -> store of the final data) is as short
    as possible.

Hardware gotchas baked in: DVE/Pool instructions may read at most ONE
PSUM operand; InstTensorTensorReduce and GpSimd-reads-from-PSUM fail
at runtime on this stack (both avoided).
"""

import os
import sys

if "/opt/trn_rl_repo" not in sys.path:
    sys.path.insert(0, "/opt/trn_rl_repo")

def _envint(name, default):
    return int(os.environ.get(name, default))

import numpy as np
import ml_dtypes

import concourse.bass as bass
import concourse.mybir as mybir
import concourse.tile as tile
from concourse import bacc
from concourse.masks import make_identity
from concourse.bass_utils import run_bass_kernel_spmd

# Problem shapes (hardcoded per harness contract)
B, N, SD, HD, AD = 512, 256, 1024, 512, 256
NCORES = 8
BL = B // NCORES          # 64 batches per core
G = 8                     # batches per group
NG = BL // G              # 8 groups
TPG = G * N // 128        # 16 row-tiles (128 hint-rows) per group
KH = HD // 128            # 4 contraction chunks over HD
EPS = 1e-8

F32 = mybir.dt.float32
BF16 = mybir.dt.bfloat16
FP8 = mybir.dt.float8e4
S_WH = 64.0     # fp8 scale on Wh.T (values ~0.02 would be subnormal)
S_Q = 32.0      # fp8 scale on q
AF = mybir.ActivationFunctionType
ALU = mybir.AluOpType
AX = mybir.AxisListType


_ACT_TABLE = "natural_log_exp_and_others"


def _patch_act_tables():
    """Force bacc's act-table chooser onto a single table that covers every
    activation this kernel uses (Square/Ln/Exp/Copy), so no per-group
    InstLoadActFuncSet reloads are emitted. Positions are preserved (the
    act_func_set_id is positional), non-chosen sets are just emptied."""
    import concourse.hw_specs as hw_specs

    orig = hw_specs.get_activation_tables

    def patched(module_arch):
        tabs = orig(module_arch)
        return {k: (v if k == _ACT_TABLE else set()) for k, v in tabs.items()}

    bacc.get_activation_tables = patched


def build_nc(stage="full"):
    """stage: 'p1'..'p12' stop after that prologue step; 'prologue';
    'proj'; 'scores'; 'softmax'; 'full'."""
    _patch_act_tables()
    if stage.startswith("p") and stage[1:].isdigit():
        cut = int(stage[1:])
    else:
        cut = 99

    nc = bacc.Bacc("TRN2", target_bir_lowering=False, debug=False,
                   num_devices=NCORES)

    wpack = nc.dram_tensor("wpack", [128, 3586], BF16,
                           kind="ExternalInput")
    wht8 = nc.dram_tensor("wht8", [128, KH, AD], FP8, kind="ExternalInput")
    xnat = nc.dram_tensor("xnat", [NG, 128, TPG, 512], BF16,
                          kind="ExternalInput")
    xnat8 = nc.dram_tensor("xnat8", [NG, 128, TPG, 512], FP8,
                           kind="ExternalInput")
    xt = nc.dram_tensor("xt", [NG, 128, TPG, KH, 128], FP8,
                        kind="ExternalInput")
    bsbh = nc.dram_tensor("bsbh", [BL, 2, AD], F32, kind="ExternalInput")
    out = nc.dram_tensor("out", [BL, HD], F32, kind="ExternalOutput")

    with tile.TileContext(nc) as tc:
        with (
            tc.tile_pool(name="singles", bufs=1) as singles,
            tc.tile_pool(name="xpool", bufs=_envint('KB_XPOOL', 4)) as xpool,
            tc.tile_pool(name="work", bufs=_envint('KB_WORK', 4)) as work,
            tc.tile_pool(name="scratch", bufs=_envint('KB_SCRATCH', 8)) as scratch,
            tc.tile_pool(name="dram", bufs=1, space="DRAM") as dram,
            tc.tile_pool(name="psz", bufs=_envint('KB_PSZ', 4), space="PSUM") as psz_pool,
            tc.tile_pool(name="psa", bufs=_envint('KB_PSA', 1), space="PSUM") as psa_pool,
            tc.tile_pool(name="pss", bufs=_envint('KB_PSS', 1), space="PSUM") as pss_pool,
            tc.tile_pool(name="psw", bufs=_envint('KB_PSW', 2), space="PSUM") as psw_pool,
        ):
            # ---------------- prologue ----------------
            ident = singles.tile([128, 128], F32)
            make_identity(nc, ident)
            ones128 = singles.tile([128, 1], F32)
            nc.vector.memset(ones128[:], 1.0)
            ones_bf = singles.tile([128, 1], BF16)
            nc.vector.memset(ones_bf[:], 1.0)

            # PE warm-up: fill the otherwise-idle startup DMA window with
            # tiny data-independent matmuls so the HAM clock gate is at
            # full rate when the first real matmuls arrive.
            n_warm = _envint('KB_WARM', 24)
            if n_warm:
                warm_ps = pss_pool.tile([128, 16], F32, tag="pt",
                                        name="warm")
                for i in range(n_warm):
                    nc.tensor.matmul(warm_ps, lhsT=ident,
                                     rhs=ident[:, 0:16],
                                     start=True, stop=True)

            # the projection's moving operand (S_WH*Wh.T in fp8) comes
            # host-prepped so the first proj matmul only waits on DMA
            whT8_sb = singles.tile([128, KH, AD], FP8)
            nc.scalar.dma_start(out=whT8_sb[:], in_=wht8[:])
            # load small tensors (wpack off the SP queue so the first data
            # loads win the DMA_ENGINES serialization race)
            wp = singles.tile([128, 3586], BF16)
            if _envint('KB_WPACK_ENG', 0):
                nc.scalar.dma_start(out=wp[:], in_=wpack[:])
            else:
                nc.sync.dma_start(out=wp[:], in_=wpack[:])
            stateT = wp[:, 0:512].rearrange("p (k b) -> p k b", b=BL)
            wsT = wp[:, 512:2560].rearrange("p (k a) -> p k a", a=AD)
            wh2_sb = wp[:, 2560:3584].rearrange("p (c h) -> p c h", h=HD)
            bh2_sb = wp[:, 3584:3586]
            bbh_sb = singles.tile([BL, 2, AD], F32)
            nc.scalar.dma_start(out=bbh_sb[:], in_=bsbh[:])
            bsb_sb = bbh_sb[:, 0, :]
            bhb_sb = bbh_sb[:, 1, :]

            def _prologue():
                if cut < 3:
                    return None

                # step 3: sp = state @ Ws.T + bs : [64, 256]
                ps_sp = psw_pool.tile([BL, AD], F32, tag="wsum",
                                      name="ps_sp")
                for k in range(SD // 128):
                    nc.tensor.matmul(ps_sp, lhsT=stateT[:, k, :],
                                     rhs=wsT[:, k, :],
                                     start=(k == 0),
                                     stop=(k == SD // 128 - 1))
                sp_sb = singles.tile([BL, AD], F32)
                nc.vector.tensor_tensor(sp_sb[:], ps_sp[:], bsb_sb[:],
                                        ALU.add)
                if cut < 4:
                    return None

                # step 4: rsn = 1 / max(|sp|, eps); spbh = <sp, bh>
                sq_sp = scratch.tile([BL, AD], F32, tag="sq_sp",
                                     name="sq_sp")
                ssq_sp = singles.tile([BL, 1], F32)
                nc.scalar.activation(out=sq_sp[:], in_=sp_sb[:],
                                     func=AF.Square)
                nc.vector.reduce_sum(out=ssq_sp[:], in_=sq_sp[:], axis=AX.X)
                # rsn = 1/max(|sp|, eps) = exp(-0.5*ln(max(ssq, eps^2)))
                # (keeps ACT on the natural_log_exp table: no table reloads)
                sn = singles.tile([BL, 1], F32)
                nc.vector.tensor_scalar_max(out=sn[:], in0=ssq_sp[:],
                                            scalar1=EPS * EPS)
                nc.scalar.activation(out=sn[:], in_=sn[:], func=AF.Ln)
                rsn = singles.tile([BL, 1], F32)
                nc.scalar.activation(out=rsn[:], in_=sn[:], func=AF.Exp,
                                     scale=-0.5)
                spbh = singles.tile([BL, 1], F32)
                dotscr = scratch.tile([BL, AD], F32, tag="sq_sp",
                                      name="dotscr")
                nc.vector.tensor_tensor(dotscr[:], sp_sb[:], bhb_sb[:],
                                        ALU.mult)
                nc.vector.reduce_sum(out=spbh[:], in_=dotscr[:], axis=AX.X)
                if cut < 5:
                    return None

                # step 5: sp.T : [128, 2, 64]
                spT = singles.tile([128, 2, BL], BF16)
                for c in range(2):
                    pst = pss_pool.tile([128, BL], F32, tag="pt",
                                        name=f"pt_sp{c}")
                    nc.tensor.transpose(
                        pst, sp_sb[:, c * 128:(c + 1) * 128],
                        ident[:BL, :BL])
                    nc.vector.tensor_copy(out=spT[:, c, :], in_=pst)
                if cut < 6:
                    return None

                # step 6: q = sp @ Wh : [64, 512]
                ps_q = psw_pool.tile([BL, HD], F32, tag="wsum",
                                     name="ps_q")
                for c in range(2):
                    nc.tensor.matmul(ps_q, lhsT=spT[:, c, :],
                                     rhs=wh2_sb[:, c, :],
                                     start=(c == 0), stop=(c == 1))
                q_sb = singles.tile([BL, HD], F32)
                nc.vector.tensor_copy(out=q_sb[:], in_=ps_q[:])
                if cut < 7:
                    return None

                # step 7: q.T (bf16) : [128, 4, 64]
                qT = singles.tile([128, KH, BL], FP8)
                for k in range(KH):
                    pst = pss_pool.tile([128, BL], F32, tag="pt",
                                        name=f"pt_q{k}")
                    nc.tensor.transpose(pst, q_sb[:, k * 128:(k + 1) * 128],
                                        ident[:BL, :BL])
                    nc.vector.tensor_scalar_mul(out=qT[:, k, :], in0=pst,
                                                scalar1=S_Q)
                if cut < 8:
                    return None

                # step 8: wb = Wh.T @ bh : [128, 4]
                ps_wb = psw_pool.tile([128, KH], F32, tag="wsum",
                                      name="ps_wb")
                for k in range(KH):
                    for c in range(2):
                        nc.tensor.matmul(
                            ps_wb[:, k:k + 1],
                            lhsT=wh2_sb[:, c, k * 128:(k + 1) * 128],
                            rhs=bh2_sb[:, c:c + 1],
                            start=(c == 0), stop=(c == 1))
                if cut < 9:
                    return None

                # step 9: |bh|^2 -> broadcast [128, 1] via DRAM roundtrip
                ps_c = psw_pool.tile([1, 1], F32, tag="wsum", name="ps_c")
                for c in range(2):
                    nc.tensor.matmul(ps_c, lhsT=bh2_sb[:, c:c + 1],
                                     rhs=bh2_sb[:, c:c + 1],
                                     start=(c == 0), stop=(c == 1))
                # scale to the S^2 domain and add the eps^2 guard so hn2's
                # Ln bias is strictly positive even if fp rounding dips the
                # norm accumulation below zero.
                c_sb = singles.tile([1, 1], F32)
                nc.vector.tensor_scalar(
                    out=c_sb[:], in0=ps_c[:], scalar1=S_WH * S_WH,
                    scalar2=EPS * EPS * S_WH * S_WH,
                    op0=ALU.mult, op1=ALU.add)
                if _envint('KB_CBCAST_MM', 1):
                    # broadcast [1,1] -> [128,1] on PE: stationary is c
                    # replicated over 128 columns (stride-0 free dim)
                    ps_cb = pss_pool.tile([128, 1], F32, tag="pt",
                                          name="ps_cb")
                    nc.tensor.matmul(ps_cb,
                                     lhsT=c_sb.to_broadcast([1, 128]),
                                     rhs=ones128[0:1, 0:1],
                                     start=True, stop=True)
                    c_bcast = singles.tile([128, 1], F32)
                    nc.vector.tensor_copy(out=c_bcast[:], in_=ps_cb[:])
                else:
                    c_dram = dram.tile([1, 1], F32)
                    nc.sync.dma_start(out=c_dram[:], in_=c_sb[:])
                    c_bcast = singles.tile([128, 1], F32)
                    nc.sync.dma_start(out=c_bcast[:],
                                      in_=c_dram.to_broadcast([128, 1]))
                if cut < 10:
                    return None

                # step 10: rsn/spbh broadcast over partitions: [128, 64, 2]
                rb_sb = singles.tile([BL, 2], F32)
                nc.vector.tensor_scalar_mul(out=rb_sb[:, 0:1], in0=rsn[:],
                                            scalar1=S_WH / S_Q)
                nc.vector.tensor_scalar_mul(out=rb_sb[:, 1:2], in0=spbh[:],
                                            scalar1=S_Q)
                rb_dram = dram.tile([BL, 2], F32)
                nc.sync.dma_start(out=rb_dram[:], in_=rb_sb[:])
                rb_bcast = singles.tile([128, BL, 2], F32)
                nc.sync.dma_start(
                    out=rb_bcast[:],
                    in_=rb_dram[None].to_broadcast([128, BL, 2]))
                if cut < 11:
                    return None

                # step 11: small aug moving operands [q_b | wb] (fp8); the
                # Wh.T part ships host-prepped (whT8_sb), so only these two
                # columns are built on-chip
                rhs_aug = []
                _nbuf = _envint('KB_NBUF', 2)
                for p in range(_nbuf):
                    buf = singles.tile([128, KH, 2], FP8,
                                       tag=f"rhsaug{p}", name=f"rhsaug{p}")
                    rhs_aug.append(buf)
                # with KB_HN2F the wb column ships pre-doubled so hn2 is
                # one tensor_tensor + Ln(bias=c) instead of ts+tt+Ln
                _wbs = (2.0 if _envint('KB_HN2F', 0) else 1.0) * S_WH * S_WH
                for p in range(_nbuf):
                    nc.vector.tensor_scalar_mul(
                        out=rhs_aug[p][:, :, 1:2],
                        in0=ps_wb[:, :, None], scalar1=_wbs)
                if cut < 12:
                    return None

                # step 12: block-diagonal attn holders (bf16)
                attn_bd = []
                for p in range(_envint('KB_ABD', 4)):
                    t = singles.tile([128, TPG, G], BF16,
                                     tag=f"attnbd{p}", name=f"attnbd{p}")
                    nc.vector.memset(t[:], 0.0)
                    attn_bd.append(t)

                return dict(qT=qT, rb_bcast=rb_bcast, c_bcast=c_bcast,
                            rhs_aug=rhs_aug, attn_bd=attn_bd)

            pro = _prologue()

            # ---------------- main loop ----------------
            if pro is not None and stage not in ("prologue",):
                qT = pro["qT"]
                rb_bcast = pro["rb_bcast"]
                c_bcast = pro["c_bcast"]
                rhs_aug = pro["rhs_aug"]
                attn_bd = pro["attn_bd"]

                pending = []
                HB = G // 2           # 4 batches per half
                HT = TPG // 2         # 8 tiles per half
                state_h = {}          # group -> dict with group tiles

                # chunk table: (g, b_lo, b_hi) in batches-within-group.
                # Normal groups process in halves; the LAST group splits
                # into quarters so its serial tail (squares -> epilogue ->
                # wsum -> store) is shorter at the drain.
                _tsplit = _envint('KB_TAILSPLIT', 4)
                chunks = []
                for g in range(NG):
                    if g == NG - 1 and _tsplit > 1:
                        step = HB // _tsplit
                        for b in range(0, G, step):
                            chunks.append((g, b, b + step))
                    else:
                        chunks.append((g, 0, HB))
                        chunks.append((g, HB, G))

                def emit_tail(chunk):
                    # PE tail of a chunk: its weighted-sum matmuls; on the
                    # group-final chunk also the normalizer and output.
                    g, b_lo, b_hi = chunk
                    st = state_h[g]
                    xn, abd, psw = (st["xn"], st["abd"], st["psw"])
                    for t in range(2 * b_lo, 2 * b_hi):
                        nc.tensor.matmul(psw, lhsT=abd[:, t, :],
                                         rhs=xn[:, t, :],
                                         start=(t == 0), stop=(t == TPG - 1),
                                         skip_group_check=True)
                    if b_hi < G:
                        return
                    # normalizer: column-sums of the whole block-diagonal
                    # (zeros off-diagonal), then per-batch sum over tiles
                    ps_se = pss_pool.tile([1, TPG * G], F32, tag="pt",
                                          name=f"ps_se{g}")
                    nc.tensor.matmul(ps_se, lhsT=ones_bf[:, 0:1],
                                     rhs=abd.rearrange("p t b -> p (t b)"),
                                     start=True, stop=True)
                    se1 = work.tile([1, G], F32, tag="se1", name=f"se1{g}")
                    nc.vector.reduce_sum(
                        out=se1[:], in_=ps_se.rearrange("p (t b) -> p b t",
                                                        b=G), axis=AX.X)
                    ps_set = pss_pool.tile([G, 1], F32, tag="pt",
                                           name=f"ps_set{g}")
                    nc.tensor.matmul(ps_set, lhsT=se1[:], rhs=ident[:1, :1],
                                     start=True, stop=True)
                    rse = work.tile([G, 1], F32, tag="rse", name=f"rse{g}")
                    nc.vector.reciprocal(out=rse[:], in_=ps_set[:])
                    outg = work.tile([G, 512], F32, tag="outg",
                                     name=f"outg{g}")
                    if _envint('KB_OUTG_POOL', 0):
                        nc.gpsimd.tensor_scalar_mul(out=outg[:], in0=psw[:],
                                                    scalar1=rse[:])
                    else:
                        nc.vector.tensor_scalar_mul(out=outg[:], in0=psw[:],
                                                    scalar1=rse[:])
                    _st = _envint('KB_STORE_ENG', 0)
                    if _st == 1:
                        nc.gpsimd.dma_start(out=out[g * G:(g + 1) * G, :],
                                            in_=outg[:])
                    elif _st == 2:
                        nc.sync.dma_start(out=out[g * G:(g + 1) * G, :],
                                          in_=outg[:])
                    else:
                        nc.scalar.dma_start(out=out[g * G:(g + 1) * G, :],
                                            in_=outg[:])
                    del state_h[g]

                for ci, chunk in enumerate(chunks):
                    g, b_lo, b_hi = chunk
                    HB_c = b_hi - b_lo
                    HT_c = 2 * HB_c
                    t_lo = 2 * b_lo
                    if b_lo == 0:
                        xtt = xpool.tile([128, TPG, KH, 128], FP8,
                                         tag="xt", name=f"xt{g}")
                        _xfp8 = g >= _envint('KB_NBF16', 0)
                        xn = xpool.tile([128, TPG, 512],
                                        FP8 if _xfp8 else BF16,
                                        tag="xnat", name=f"xn{g}")
                        ssq_g = work.tile([128, TPG], F32, tag="ssq",
                                          name=f"ssq{g}")
                        abd = attn_bd[g % len(attn_bd)]
                        psw = psw_pool.tile([G, 512], F32, tag="wsum",
                                            name=f"psw{g}")
                        state_h[g] = dict(xtt=xtt, xn=xn, ssq=ssq_g,
                                          abd=abd, psw=psw)
                    st = state_h[g]
                    xtt, xn, ssq_g, abd = (
                        st["xtt"], st["xn"], st["ssq"], st["abd"])
                    # per-chunk aug outputs: columns 2t'   = zq' (S_Q*<q,x>)
                    #                               2t'+1 = zw' (S^2*<x,wb>)
                    ps_augh = psa_pool.tile([128, 2 * HT_c], F32, tag="aug",
                                            name=f"aug{ci}")

                    # loads: per group (b_lo==0) or per chunk, QSPLIT pieces
                    if _envint('KB_LOADG', 0):
                        if b_lo == 0:
                            nc.sync.dma_start(out=xtt[:], in_=xt[g][:])
                            nc.sync.dma_start(
                                out=xn[:],
                                in_=(xnat8 if _xfp8 else xnat)[g][:])
                    else:
                        _qs = _envint('KB_QSPLIT', 2) if HT_c == HT else 1
                        _step = HT_c // _qs
                        for _q in range(_qs):
                            _a = t_lo + _q * _step
                            nc.sync.dma_start(out=xtt[:, _a:_a + _step],
                                              in_=xt[g][:, _a:_a + _step])
                        _xsrc = xnat8 if _xfp8 else xnat
                        for _q in range(_qs):
                            _a = t_lo + _q * _step
                            nc.sync.dma_start(out=xn[:, _a:_a + _step],
                                              in_=_xsrc[g][:, _a:_a + _step])

                    # projection for this chunk's batches
                    for bl in range(b_lo, b_hi):
                        b = g * G + bl
                        buf = rhs_aug[b % len(rhs_aug)]
                        nc.gpsimd.tensor_copy(out=buf[:, :, 0:1],
                                              in_=qT[:, :, b:b + 1])
                        for t2 in range(2):
                            t = bl * 2 + t2
                            tp = t - t_lo
                            psz = psz_pool.tile([128, AD], F32, tag="z",
                                                name=f"z{g}_{t}")
                            for k2 in range(KH // 2):
                                nc.tensor.matmul(
                                    psz,
                                    lhsT=xtt[:, t, 2 * k2:2 * k2 + 2, :],
                                    rhs=whT8_sb[:, 2 * k2:2 * k2 + 2, :],
                                    start=(k2 == 0),
                                    stop=(k2 == KH // 2 - 1),
                                    perf_mode=mybir.MatmulPerfMode.DoubleRow)
                            for k2 in range(KH // 2):
                                nc.tensor.matmul(
                                    ps_augh[:, 2 * tp:2 * tp + 2],
                                    lhsT=xtt[:, t, 2 * k2:2 * k2 + 2, :],
                                    rhs=buf[:, 2 * k2:2 * k2 + 2, :],
                                    start=(k2 == 0),
                                    stop=(k2 == KH // 2 - 1),
                                    perf_mode=mybir.MatmulPerfMode.DoubleRow)
                            sq = scratch.tile([128, AD], BF16, tag="sq",
                                              name=f"sq{g}_{t}")
                            # squares produce RAW sum(z'^2) (z' = S_WH*z);
                            # the 1/S^2 is folded into the hn2 constants.
                            # ACT squares (no accum-read), DVE reduces.
                            nc.scalar.activation(
                                out=sq[:], in_=psz[:],
                                func=AF.Square, scale=1.0)
                            nc.vector.reduce_sum(
                                out=ssq_g[:, t:t + 1], in_=sq[:],
                                axis=AX.X)

                    if stage == "proj":
                        continue

                    # ---- epilogue for this chunk: norms and scores ----
                    hs = slice(t_lo, t_lo + HT_c)
                    if _envint('KB_AUGCP', 1):
                        # free the aug PSUM bank early: one copy to SBUF,
                        # epilogue reads the SBUF mirror
                        augs = work.tile([128, 2 * HT_c], F32, tag="augs",
                                         name=f"augs{ci}")
                        nc.vector.tensor_copy(out=augs[:], in_=ps_augh[:])
                        ps_augh = augs
                    # hn2 is computed in the S_WH^2-scaled domain (raw
                    # sum(z'^2) + 2*zw' + S^2*(|bh|^2+eps^2) = S^2*|hp|^2);
                    # the S factor is folded into rb_sb[:,0] (rsn*S_WH/S_Q)
                    # and the constant term rides the Ln bias.
                    hn2 = work.tile([128, HT_c], F32, tag="hn2",
                                    name=f"hn2_{ci}")
                    if _envint('KB_HN2F', 0):
                        # wb column ships pre-doubled: hn2 = zw'' + ssq,
                        # the constant rides the Ln bias
                        nc.vector.tensor_tensor(hn2[:],
                                                ps_augh[:, 1:2 * HT_c:2],
                                                ssq_g[:, hs], ALU.add)
                        nc.scalar.activation(out=hn2[:], in_=hn2[:],
                                             func=AF.Ln, bias=c_bcast[:])
                    elif _envint('KB_HN2OLD', 1):
                        nc.vector.tensor_scalar(
                            out=hn2[:], in0=ps_augh[:, 1:2 * HT_c:2],
                            scalar1=2.0, scalar2=c_bcast[:],
                            op0=ALU.mult, op1=ALU.add)
                        nc.vector.tensor_tensor(hn2[:], hn2[:],
                                                ssq_g[:, hs], ALU.add)
                        nc.scalar.activation(out=hn2[:], in_=hn2[:],
                                             func=AF.Ln)
                    else:
                        nc.vector.scalar_tensor_tensor(
                            out=hn2[:], in0=ps_augh[:, 1:2 * HT_c:2],
                            scalar=2.0, in1=ssq_g[:, hs],
                            op0=ALU.mult, op1=ALU.add)
                        # rhn = 1/|hp| = exp(-0.5*ln(hn2 + c))
                        nc.scalar.activation(out=hn2[:], in_=hn2[:],
                                             func=AF.Ln, bias=c_bcast[:])
                    rhn = work.tile([128, HT_c], F32, tag="rhn",
                                    name=f"rhn{ci}")
                    nc.scalar.activation(out=rhn[:], in_=hn2[:], func=AF.Exp,
                                         scale=-0.5)

                    scores = work.tile([128, HB_c, 2], F32, tag="scores",
                                       name=f"scores{ci}")
                    zq_v = ps_augh[:, 0:2 * HT_c:2].rearrange(
                        "p (b h) -> p b h", h=2)
                    rhn_v = rhn.rearrange("p (b h) -> p b h", h=2)
                    b0 = g * G + b_lo
                    spbh_rep = rb_bcast[:, b0:b0 + HB_c, 1:2].to_broadcast(
                        [128, HB_c, 2])
                    rsn_rep = rb_bcast[:, b0:b0 + HB_c, 0:1].to_broadcast(
                        [128, HB_c, 2])
                    _se = nc.gpsimd if _envint('KB_SCPOOL', 0) else \
                        nc.vector
                    _se.tensor_tensor(scores[:], zq_v, spbh_rep,
                                      ALU.add)
                    _se.tensor_tensor(scores[:], scores[:], rsn_rep,
                                      ALU.mult)
                    _se.tensor_tensor(scores[:], scores[:], rhn_v,
                                      ALU.mult)

                    if stage == "scores":
                        continue

                    # exp(scores): cosine sims are in [-1,1], no
                    # max-subtraction needed. The exp writes STRAIGHT into
                    # the block-diagonal stationary (strided out AP), so no
                    # separate scatter copies; the normalizer later reads
                    # column-sums of the whole block-diagonal.
                    abd_flat = abd.rearrange("p t b -> p (t b)")
                    stride = 2 * G + 1
                    for t2 in range(2):
                        s0 = b_lo * stride + t2 * G
                        nc.scalar.activation(
                            out=abd_flat[:, s0:s0 + (HB_c - 1) * stride
                                         + 1:stride],
                            in_=scores[:, :, t2], func=AF.Exp)

                    pending.append(chunk)
                    _pend = _envint('KB_PEND', 4)
                    if ci >= len(chunks) - _envint('KB_PENDTAIL', 5):
                        _pend = _envint('KB_PENDMIN', 0)
                    while len(pending) > _pend:
                        emit_tail(pending.pop(0))

                while pending:
                    emit_tail(pending.pop(0))

    nc.compile()
    return nc


_NC = None


def _get_nc():
    global _NC
    if _NC is None:
        _NC = build_nc()
    return _NC


def _diffuse_fp8(x):
    """Error-diffusion fp8 quantization along the hint-row axis (per
    batch): q_n = fp8(x_n + carry), carry += x_n - q_n. The weighted sum
    averages ~N rows with near-uniform softmax weights, so consecutive
    quantization errors telescope -- ~9x lower wsum error than
    round-to-nearest at the same bit-width."""
    fp8 = ml_dtypes.float8_e4m3
    q = np.empty(x.shape, fp8)
    carry = np.zeros((x.shape[0], x.shape[2]), np.float32)
    for n in range(x.shape[1]):
        v = x[:, n, :] + carry
        qn = v.astype(fp8)
        carry = v - qn.astype(np.float32)
        q[:, n, :] = qn
    return q


def _prep_core_inputs(state_emb, hints_emb, Ws, bs, Wh, bh, core,
                      hints_d8):
    bf16 = ml_dtypes.bfloat16
    s = slice(core * BL, (core + 1) * BL)
    hf = np.ascontiguousarray(hints_emb[s]).reshape(BL * N, HD)
    hfb = hf.astype(bf16)
    hf8 = hf.astype(ml_dtypes.float8_e4m3)
    hf8d = hints_d8[s].reshape(BL * N, HD)
    # natural: (g, p, t, f) with row = g*2048 + t*128 + p
    xnat = np.ascontiguousarray(
        hfb.reshape(NG, TPG, 128, 512).transpose(0, 2, 1, 3))
    xnat8 = np.ascontiguousarray(
        hf8d.reshape(NG, TPG, 128, 512).transpose(0, 2, 1, 3))
    # transposed: (g, p, t, k, r) with row = g*2048 + t*128 + r, h = k*128+p
    xtd = np.ascontiguousarray(
        hf8.reshape(NG, TPG, 128, KH, 128).transpose(0, 4, 1, 3, 2))
    bf = ml_dtypes.bfloat16
    # Ws.T arranged [s_in_chunk, s_chunk, a]
    wst = Ws.T.reshape(SD // 128, 128, AD).transpose(1, 0, 2)
    wh2 = Wh.reshape(2, 128, HD).transpose(1, 0, 2)
    # S_WH*Wh.T arranged [h_in_chunk, h_chunk, a], fp8 (proj moving operand)
    wht8 = np.ascontiguousarray(
        (Wh.T.reshape(KH, 128, AD).transpose(1, 0, 2) * S_WH)
    ).astype(ml_dtypes.float8_e4m3)
    bh2 = bh.reshape(2, 128).T
    bsbh = np.ascontiguousarray(np.stack([
        np.broadcast_to(bs, (BL, AD)),
        np.broadcast_to(bh, (BL, AD)),
    ], axis=1)).astype(np.float32)
    st = np.asarray(state_emb[s])
    # state.T arranged [s_in_chunk, s_chunk, b]
    statet = st.T.reshape(SD // 128, 128, BL).transpose(1, 0, 2)
    wpack = np.concatenate([
        statet.reshape(128, -1), wst.reshape(128, -1),
        wh2.reshape(128, -1),
        bh2.reshape(128, -1),
    ], axis=1).astype(bf)
    wpack = np.ascontiguousarray(wpack)
    return {
        "wpack": wpack,
        "wht8": wht8,
        "xnat": xnat,
        "xnat8": xnat8,
        "xt": xtd,
        "bsbh": bsbh,
    }


def kernel(state_emb, hints_emb, Ws, bs, Wh, bh):
    state_emb = np.asarray(state_emb, dtype=np.float32)
    hints_emb = np.asarray(hints_emb, dtype=np.float32)
    Ws = np.asarray(Ws, dtype=np.float32)
    bs = np.asarray(bs, dtype=np.float32)
    Wh = np.asarray(Wh, dtype=np.float32)
    bh = np.asarray(bh, dtype=np.float32)

    nc = _get_nc()
    hints_d8 = _diffuse_fp8(hints_emb)
    in_maps = [
        _prep_core_inputs(state_emb, hints_emb, Ws, bs, Wh, bh, c,
                          hints_d8)
        for c in range(NCORES)
    ]
    res = run_bass_kernel_spmd(nc, in_maps, core_ids=list(range(NCORES)))
    return np.concatenate([res.results[c]["out"] for c in range(NCORES)],
                          axis=0)



# revision 19
# speedup vs baseline: 1.2512x; 1.2512x over previous
"""Trainium2 Bass kernel for cosine-similarity hint attention.

Computation (per batch b):
  sp = state_emb @ Ws.T + bs                  (B, A)
  hp = hints_emb @ Wh.T + bh                  (B, N, A)
  scores = <sp, hp> / (max(|sp|,eps) * max(|hp|,eps))
  attn = softmax(scores, axis=N)
  out = attn @ hints_emb                      (B, HD)

Distribution: data-parallel over batch, B=512 -> 64 per core on 8 cores.
Weights replicated. No collectives.

Device-side structure (per core, 8 groups of 8 batches = 16 row-tiles of
128 hint-rows each):
  - hints ship in two host-prepared fp8-e4m3 layouts: transposed [h, r]
    (proj/aug stationaries, DoubleRow pairs on k-chunks) and natural
    [r, h] (weighted-sum stationaries). The natural copy is quantized
    with error diffusion along the hint-row axis (errors telescope in
    the softmax-weighted average). One big DMA per (group, layout)
    keeps the DMA stream dense -- the kernel is DMA-bound and every
    other engine runs well under the DMA transfer time.
  - hint projection z' = X @ (S*Wh.T) on TensorE in fp8 DoubleRow mode;
    a second tiny DoubleRow matmul per tile drops zq' = S_Q*<q_b,x_r>
    and zw' = 2*S^2*<x_r,wb> into a per-group [128, 16, 2] PSUM tile.
  - |hp|^2 via a per-pair split across engines: bn_stats on DVE (one
    pass over 2 tiles of PSUM -> even/odd mean+var -> ssq), or ACT
    Square -> SBUF bf16 + GpSimd tensor_scalar accum (free-dim sum) --
    tunable split so ACT/DVE/Pool all stay under the DMA roofline.
  - softmax needs no max-subtraction (cosine scores are in [-1,1]);
    exp(scores) writes a dense per-group [128, 16] bf16 attn tile
    (col = 2*b_in_group + half), contiguous with the scores layout.
  - the weighted sum runs TRANSPOSED: out.T[h, b] += xnat_chunk.T @
    attn_col, one matmul per (tile, h-chunk) with a 1-wide moving
    operand, all accumulating into a single persistent PSUM bank.
    The per-group softmax normalizer (ones-matmul column sums) is
    broadcast across partitions with a stride-0 ones-row matmul (no
    DRAM roundtrip), and each group's out.T slice is normalized and
    stored as soon as its matmuls land. The host undoes the output
    transpose (layout-only).

Hardware gotchas baked in: DVE/Pool instructions may read at most ONE
PSUM operand; InstTensorTensorReduce and GpSimd-reads-from-PSUM fail
at runtime on this stack (both avoided).
"""

import os
import sys

if "/opt/trn_rl_repo" not in sys.path:
    sys.path.insert(0, "/opt/trn_rl_repo")


def _envint(name, default):
    return int(os.environ.get(name, default))


import numpy as np
import ml_dtypes

import concourse.bass as bass
import concourse.mybir as mybir
import concourse.tile as tile
from concourse import bacc
from concourse.masks import make_identity
from concourse.bass_utils import run_bass_kernel_spmd

# Problem shapes (hardcoded per harness contract)
B, N, SD, HD, AD = 512, 256, 1024, 512, 256
NCORES = 8
BL = B // NCORES          # 64 batches per core
G = 8                     # batches per group
NG = BL // G              # 8 groups
TPG = G * N // 128        # 16 row-tiles (128 hint-rows) per group
KH = HD // 128            # 4 contraction chunks over HD
KC = HD // 128            # h-chunks for the transposed weighted sum
EPS = 1e-8

F32 = mybir.dt.float32
BF16 = mybir.dt.bfloat16
FP8 = mybir.dt.float8e4
S_WH = 64.0     # fp8 scale on Wh.T (values ~0.02 would be subnormal)
S_Q = 32.0      # fp8 scale on q
AF = mybir.ActivationFunctionType
ALU = mybir.AluOpType
AX = mybir.AxisListType


_ACT_TABLE = "natural_log_exp_and_others"


def _patch_act_tables():
    """Force bacc's act-table chooser onto a single table that covers every
    activation this kernel uses (Square/Ln/Exp/Copy), so no per-group
    InstLoadActFuncSet reloads are emitted."""
    import concourse.hw_specs as hw_specs

    orig = hw_specs.get_activation_tables

    def patched(module_arch):
        tabs = orig(module_arch)
        return {k: (v if k == _ACT_TABLE else set()) for k, v in tabs.items()}

    bacc.get_activation_tables = patched


def build_nc(stage="full"):
    _patch_act_tables()

    nc = bacc.Bacc("TRN2", target_bir_lowering=False, debug=False,
                   num_devices=NCORES)

    wpack = nc.dram_tensor("wpack", [128, 3586], BF16, kind="ExternalInput")
    wht8 = nc.dram_tensor("wht8", [128, KH, AD], FP8, kind="ExternalInput")
    xnat8 = nc.dram_tensor("xnat8", [NG, 128, TPG, 512], FP8,
                           kind="ExternalInput")
    xt = nc.dram_tensor("xt", [NG, 128, TPG, KH, 128], FP8,
                        kind="ExternalInput")
    bsb = nc.dram_tensor("bsb", [BL, AD], F32, kind="ExternalInput")
    # transposed output: outT[p, c, b] = out[b, c*128 + p]; host transposes
    outT = nc.dram_tensor("outT", [128, KC, BL], F32, kind="ExternalOutput")
    DBG = _envint("KB_DBG", 0)
    if DBG:
        dbg = nc.dram_tensor("dbg", [128, 16, 8], F32, kind="ExternalOutput")

    # how many tile-pairs per group go down the bn_stats (DVE) path; the
    # rest take ACT Square -> SBUF + GpSimd accum.
    NBN = _envint("KB_NBN", 4)          # of 8 pairs per group
    # interleave the two ssq paths so consecutive psz frees alternate
    # between DVE (bn_stats) and ACT+Pool
    _pat = [False, True, False, True, False, True, True, True]
    BN_PAIRS = set(i for i in range(8) if _pat[i]) if NBN == 5 else         set(range(NBN))
    XPB = _envint("KB_XPB", 3)          # hint buffers (per layout)

    with tile.TileContext(nc) as tc:
        with (
            tc.tile_pool(name="singles", bufs=1) as singles,
            tc.tile_pool(name="xpool", bufs=XPB) as xpool,
            tc.tile_pool(name="work", bufs=_envint("KB_WORK", 4)) as work,
            tc.tile_pool(name="psz", bufs=_envint("KB_PSZ", 5),
                         space="PSUM") as psz_pool,
            tc.tile_pool(name="psa", bufs=_envint("KB_PSA", 1),
                         space="PSUM") as psa_pool,
            tc.tile_pool(name="pso", bufs=1, space="PSUM") as pso_pool,
            tc.tile_pool(name="pst", bufs=1, space="PSUM") as pst_pool,
        ):
            # ---------------- constants ----------------
            ident = singles.tile([128, 128], F32)
            make_identity(nc, ident)
            ones128 = singles.tile([128, 1], F32)
            nc.vector.memset(ones128[:], 1.0)
            ones_bf = singles.tile([128, 1], BF16)
            nc.vector.memset(ones_bf[:], 1.0)
            one_f32 = singles.tile([1, 1], F32)
            nc.vector.memset(one_f32[:], 1.0)

            # PE warm-up: fill the startup DMA window with tiny matmuls so
            # the p-state ramp is done before the first real matmuls.
            n_warm = _envint("KB_WARM", 24)
            if n_warm:
                warm_ps = pst_pool.tile([128, 16], F32, tag="pt",
                                        name="warm")
                for i in range(n_warm):
                    nc.tensor.matmul(warm_ps, lhsT=ident,
                                     rhs=ident[:, 0:16],
                                     start=True, stop=True)

            # ---------------- DMA stream ----------------
            # hint tiles; first group's transposed load split for a faster
            # pipeline lead-in, wpack interleaved after the first half.
            xtt_t = []
            xn_t = []
            for g in range(NG):
                xtt_t.append(xpool.tile([128, TPG, KH, 128], FP8, tag="xt",
                                        name=f"xt{g}"))
                xn_t.append(xpool.tile([128, TPG, 512], FP8, tag="xnat",
                                       name=f"xn{g}"))
            HT = TPG // 2
            nc.sync.dma_start(out=xtt_t[0][:, 0:HT], in_=xt[0][:, 0:HT])
            whT8_sb = singles.tile([128, KH, AD], FP8)
            nc.scalar.dma_start(out=whT8_sb[:], in_=wht8[:])
            wp = singles.tile([128, 3586], BF16)
            nc.sync.dma_start(out=wp[:], in_=wpack[:])
            bsb_sb = singles.tile([BL, AD], F32)
            nc.scalar.dma_start(out=bsb_sb[:], in_=bsb[:])
            nc.sync.dma_start(out=xtt_t[0][:, HT:TPG], in_=xt[0][:, HT:TPG])
            nc.sync.dma_start(out=xn_t[0][:], in_=xnat8[0][:])
            for g in range(1, NG):
                nc.sync.dma_start(out=xtt_t[g][:], in_=xt[g][:])
                nc.sync.dma_start(out=xn_t[g][:], in_=xnat8[g][:])

            stateT = wp[:, 0:512].rearrange("p (k b) -> p k b", b=BL)
            wsT = wp[:, 512:2560].rearrange("p (k a) -> p k a", a=AD)
            wh2_sb = wp[:, 2560:3584].rearrange("p (c h) -> p c h", h=HD)
            bh2_sb = wp[:, 3584:3586]

            # ---------------- prologue (sp / q / constants) ----------------
            # sp = state @ Ws.T + bs : [64, 256]
            ps_sp = psa_pool.tile([BL, AD], F32, tag="aug", name="ps_sp")
            for k in range(SD // 128):
                nc.tensor.matmul(ps_sp, lhsT=stateT[:, k, :],
                                 rhs=wsT[:, k, :],
                                 start=(k == 0),
                                 stop=(k == SD // 128 - 1))
            sp_sb = singles.tile([BL, AD], F32)
            nc.vector.tensor_tensor(sp_sb[:], ps_sp[:], bsb_sb[:], ALU.add)

            # rsn = 1/max(|sp|, eps) = exp(-0.5*ln(max(ssq, eps^2)))
            sq_sp = work.tile([BL, AD], F32, tag="sq", name="sq_sp")
            ssq_sp = singles.tile([BL, 1], F32)
            nc.scalar.activation(out=sq_sp[:], in_=sp_sb[:], func=AF.Square)
            nc.vector.reduce_sum(out=ssq_sp[:], in_=sq_sp[:], axis=AX.X)
            sn = singles.tile([BL, 1], F32)
            nc.vector.tensor_scalar_max(out=sn[:], in0=ssq_sp[:],
                                        scalar1=EPS * EPS)
            nc.scalar.activation(out=sn[:], in_=sn[:], func=AF.Ln)
            rsn = singles.tile([BL, 1], F32)
            nc.scalar.activation(out=rsn[:], in_=sn[:], func=AF.Exp,
                                 scale=-0.5)

            # sp.T : [128, 2, 64]
            spT = singles.tile([128, 2, BL], BF16)
            for c in range(2):
                pstt = pst_pool.tile([128, BL], F32, tag="pt",
                                     name=f"pt_sp{c}")
                nc.tensor.transpose(pstt, sp_sb[:, c * 128:(c + 1) * 128],
                                    ident[:BL, :BL])
                nc.vector.tensor_copy(out=spT[:, c, :], in_=pstt)

            # q = sp @ Wh : [64, 512]
            ps_q = psa_pool.tile([BL, HD], F32, tag="aug", name="ps_q")
            for c in range(2):
                nc.tensor.matmul(ps_q, lhsT=spT[:, c, :],
                                 rhs=wh2_sb[:, c, :],
                                 start=(c == 0), stop=(c == 1))
            q_sb = singles.tile([BL, HD], F32)
            nc.vector.tensor_copy(out=q_sb[:], in_=ps_q[:])

            # spbh row [1, 64] = bh.T @ sp.T  (contraction over a-chunks)
            ps_sdr = pst_pool.tile([1, BL], F32, tag="pt", name="ps_sdr")
            for c in range(2):
                nc.tensor.matmul(ps_sdr, lhsT=bh2_sb[:, c:c + 1],
                                 rhs=spT[:, c, :],
                                 start=(c == 0), stop=(c == 1))
            rowbuf = singles.tile([1, 2, BL], F32)
            nc.vector.tensor_scalar_mul(out=rowbuf[:, 1, :], in0=ps_sdr[:],
                                        scalar1=S_Q)

            # rsn row [1, 64] via PE transpose
            ps_rr = pst_pool.tile([1, BL], F32, tag="pt", name="ps_rr")
            nc.tensor.transpose(ps_rr, rsn[:], ident[:BL, :BL])
            nc.vector.tensor_scalar_mul(out=rowbuf[:, 0, :], in0=ps_rr[:],
                                        scalar1=S_WH / S_Q)

            # broadcast both rows over partitions: [128, 2, 64]
            ps_rb = pst_pool.tile([128, 2, BL], F32, tag="pt", name="ps_rb")
            for a in range(2):
                nc.tensor.matmul(
                    ps_rb[:, a], lhsT=one_f32.to_broadcast([1, 128]),
                    rhs=rowbuf[:, a, :], start=True, stop=True)
            rb_bcast = singles.tile([128, 2, BL], F32)
            nc.vector.tensor_copy(out=rb_bcast[:], in_=ps_rb[:])

            # per-batch aug moving operands [S_Q*q_b | 2*S^2*wb] (fp8):
            # aug_all[:, k, b, 0] = S_Q * q.T, aug_all[:, k, b, 1] = wb col
            aug_all = singles.tile([128, KH, BL, 2], FP8)
            for k in range(KH):
                pstt = pst_pool.tile([128, BL], F32, tag="pt",
                                     name=f"pt_q{k}")
                nc.tensor.transpose(pstt, q_sb[:, k * 128:(k + 1) * 128],
                                    ident[:BL, :BL])
                nc.vector.tensor_scalar_mul(out=aug_all[:, k, :, 0],
                                            in0=pstt, scalar1=S_Q)

            # wb = Wh.T @ bh : [128, 4]
            ps_wb = pst_pool.tile([128, KH], F32, tag="pt", name="ps_wb")
            for k in range(KH):
                for c in range(2):
                    nc.tensor.matmul(
                        ps_wb[:, k:k + 1],
                        lhsT=wh2_sb[:, c, k * 128:(k + 1) * 128],
                        rhs=bh2_sb[:, c:c + 1],
                        start=(c == 0), stop=(c == 1))

            # c = S^2*(|bh|^2 + eps^2), broadcast [128, 1] via PE
            ps_c = pst_pool.tile([1, 1], F32, tag="pt", name="ps_c")
            for c in range(2):
                nc.tensor.matmul(ps_c, lhsT=bh2_sb[:, c:c + 1],
                                 rhs=bh2_sb[:, c:c + 1],
                                 start=(c == 0), stop=(c == 1))
            c_sb = singles.tile([1, 1], F32)
            nc.vector.tensor_scalar(
                out=c_sb[:], in0=ps_c[:], scalar1=S_WH * S_WH,
                scalar2=EPS * EPS * S_WH * S_WH,
                op0=ALU.mult, op1=ALU.add)
            ps_cb = pst_pool.tile([128, 1], F32, tag="pt", name="ps_cb")
            nc.tensor.matmul(ps_cb, lhsT=c_sb.to_broadcast([1, 128]),
                             rhs=one_f32[:], start=True, stop=True)
            c_bcast = singles.tile([128, 1], F32)
            nc.vector.tensor_copy(out=c_bcast[:], in_=ps_cb[:])

            # wb column, broadcast over batches
            nc.vector.tensor_scalar_mul(
                out=aug_all[:, :, :, 1],
                in0=ps_wb[:, :, None].to_broadcast([128, KH, BL]),
                scalar1=2.0 * S_WH * S_WH)

            # persistent transposed-output accumulator [128, KC, BL]
            otp = pso_pool.tile([128, KC, BL], F32, tag="oT", name="otp")

            # ---------------- main loop ----------------
            if stage != "prologue":
                st_g = {}

                def emit_front(g, plo=0, phi=TPG // 2):
                    if plo == 0:
                        ps_aug = psa_pool.tile([128, TPG, 2], F32,
                                               tag="aug", name=f"aug{g}")
                        ssq_g = work.tile([128, TPG], F32, tag="ssq",
                                          name=f"ssq{g}")
                        stats_g = work.tile([128, TPG, 6], F32, tag="stats",
                                            name=f"stats{g}")
                        st_g[g] = (ps_aug, ssq_g, stats_g)
                    xtt = xtt_t[g]
                    ps_aug, ssq_g, stats_g = st_g[g]

                    # projection + ssq, one pair of tiles at a time
                    for p8 in range(plo, phi):
                        psz = psz_pool.tile([128, 2, AD], F32, tag="z",
                                            name=f"z{g}_{p8}")
                        for t2 in range(2):
                            t = 2 * p8 + t2
                            for k2 in range(KH // 2):
                                nc.tensor.matmul(
                                    psz[:, t2],
                                    lhsT=xtt[:, t, 2 * k2:2 * k2 + 2, :],
                                    rhs=whT8_sb[:, 2 * k2:2 * k2 + 2, :],
                                    start=(k2 == 0),
                                    stop=(k2 == KH // 2 - 1),
                                    perf_mode=mybir.MatmulPerfMode.DoubleRow)

                        if p8 in BN_PAIRS:
                            # DVE path: per-tile BNStats (HW requires out
                            # exactly [P, 6]), then
                            # ssq = cv_e + cv_o + 128*(m_e^2 + m_o^2)
                            st2 = stats_g[:, 2 * p8:2 * p8 + 2, :]
                            for t2 in range(2):
                                nc.vector.bn_stats(
                                    out=stats_g[:, 2 * p8 + t2, :],
                                    in_=psz[:, t2])
                            msq = work.tile([128, 2, 2], F32, tag="m2",
                                            name=f"m2_{g}_{p8}")
                            if _envint("KB_BNPOOL", 1):
                                # Pool: msq = m*m, *128, + cnt*var (plain
                                # TensorTensor/TensorScalar only)
                                nc.gpsimd.tensor_tensor(
                                    msq[:], st2[:, :, 1::3], st2[:, :, 1::3],
                                    ALU.mult)
                                nc.gpsimd.tensor_scalar_mul(
                                    out=msq[:], in0=msq[:], scalar1=128.0)
                                nc.gpsimd.tensor_tensor(
                                    msq[:], msq[:], st2[:, :, 2::3], ALU.add)
                            else:
                                nc.vector.tensor_tensor(
                                    msq[:], st2[:, :, 1::3], st2[:, :, 1::3],
                                    ALU.mult)
                                nc.vector.scalar_tensor_tensor(
                                    out=msq[:], in0=msq[:], scalar=128.0,
                                    in1=st2[:, :, 2::3], op0=ALU.mult,
                                    op1=ALU.add)
                            nc.vector.reduce_sum(
                                out=ssq_g[:, 2 * p8:2 * p8 + 2],
                                in_=msq[:], axis=AX.X)
                        else:
                            # ACT path: in-place Square with accumulator
                            for t2 in range(2):
                                nc.scalar.activation(
                                    out=psz[:, t2], in_=psz[:, t2],
                                    func=AF.Square,
                                    accum_out=ssq_g[:, 2 * p8 + t2:
                                                    2 * p8 + t2 + 1])

                    # aug matmuls for this chunk
                    for p8 in range(plo, phi):
                        b = g * G + p8
                        for t2 in range(2):
                            t = 2 * p8 + t2
                            for k2 in range(KH // 2):
                                nc.tensor.matmul(
                                    ps_aug[:, t, :],
                                    lhsT=xtt[:, t, 2 * k2:2 * k2 + 2, :],
                                    rhs=aug_all[:, 2 * k2:2 * k2 + 2, b, :],
                                    start=(k2 == 0),
                                    stop=(k2 == KH // 2 - 1),
                                    perf_mode=mybir.MatmulPerfMode.DoubleRow)

                at_g = {}
                wsum_n = [0]

                def emit_tail1(g, plo=0, phi=TPG // 2):
                    ps_aug, ssq_g, _ = st_g[g]
                    if phi == TPG // 2:
                        st_g.pop(g)
                    xn = xn_t[g]
                    NB = phi - plo                   # batches in this chunk
                    tlo, thi = 2 * plo, 2 * phi      # tile range
                    ts_ = slice(tlo, thi)
                    attn_g = work.tile([128, 2 * NB], BF16, tag="attn",
                                       name=f"attn{g}_{plo}")
                    at_g[(g, plo)] = (attn_g, NB, phi)

                    # ---- chunk epilogue: norms, scores, exp ----
                    hn2 = work.tile([128, 2 * NB], F32, tag="hn2",
                                    name=f"hn2_{g}_{plo}")
                    nc.vector.scalar_tensor_tensor(
                        out=hn2[:], in0=ssq_g[:, ts_], scalar=1.0,
                        in1=ps_aug[:, ts_, 1], op0=ALU.mult, op1=ALU.add)
                    nc.scalar.activation(out=hn2[:], in_=hn2[:], func=AF.Ln,
                                         bias=c_bcast[:])
                    rhn = work.tile([128, 2 * NB], F32, tag="rhn",
                                    name=f"rhn{g}_{plo}")
                    nc.scalar.activation(out=rhn[:], in_=hn2[:], func=AF.Exp,
                                         scale=-0.5)

                    scores = work.tile([128, NB, 2], F32, tag="scores",
                                       name=f"scores{g}_{plo}")
                    b0 = g * G + plo
                    spbh_rep = rb_bcast[:, 1, b0:b0 + NB][:, :, None] \
                        .to_broadcast([128, NB, 2])
                    rsn_rep = rb_bcast[:, 0, b0:b0 + NB][:, :, None] \
                        .to_broadcast([128, NB, 2])
                    nc.vector.scalar_tensor_tensor(
                        out=scores[:], in0=ps_aug[:, ts_, 0].rearrange(
                            "p (b h) -> p b h", h=2),
                        scalar=1.0, in1=spbh_rep, op0=ALU.mult, op1=ALU.add)
                    nc.vector.tensor_tensor(scores[:], scores[:], rsn_rep,
                                            ALU.mult)
                    nc.vector.tensor_tensor(
                        scores[:], scores[:],
                        rhn.rearrange("p (b h) -> p b h", h=2), ALU.mult)

                    # exp(scores) -> attn (dense, col = 2*b_in_chunk + t2)
                    nc.scalar.activation(
                        out=attn_g.rearrange("p (b h) -> p b h", h=2),
                        in_=scores[:], func=AF.Exp)
                    if DBG and g == 0 and plo == 0:
                        nc.sync.dma_start(out=dbg[:, :, 0], in_=ssq_g[:])
                        nc.sync.dma_start(out=dbg[:, :, 1], in_=hn2[:])
                        nc.sync.dma_start(out=dbg[:, :, 2], in_=rhn[:])
                        nc.sync.dma_start(
                            out=dbg[:, :, 3],
                            in_=scores.rearrange("p b h -> p (b h)"))
                        dbga = work.tile([128, TPG], F32, tag="dbga",
                                         name="dbga")
                        nc.vector.tensor_copy(out=dbga[:], in_=attn_g[:])
                        nc.sync.dma_start(out=dbg[:, :, 4], in_=dbga[:])
                        nc.sync.dma_start(out=dbg[:, 0:2, 5],
                                          in_=rb_bcast[:, :, 0])
                        dbgq = work.tile([128, TPG, 2], F32, tag="dbgq",
                                         name="dbgq")
                        nc.vector.tensor_copy(out=dbgq[:], in_=ps_aug[:])
                        nc.sync.dma_start(out=dbg[:, :, 6], in_=dbgq[:, :, 0])
                        nc.sync.dma_start(out=dbg[:, :, 7], in_=dbgq[:, :, 1])

                    # ---- transposed weighted sum ----
                    # PSUM pending-zero granularity is the whole 2KB bank:
                    # start=True ONLY on the very first matmul of the whole
                    # accumulation (marks the bank pending-zero; each later
                    # matmul overwrites-on-first-touch then accumulates),
                    # stop=True only on the very last.
                    for t in range(tlo, thi):
                        gb = g * G + t // 2
                        for c in range(KC):
                            wsum_n[0] += 1
                            nc.tensor.matmul(
                                otp[:, c, gb:gb + 1],
                                lhsT=xn[:, t, 128 * c:128 * (c + 1)],
                                rhs=attn_g[:, t - tlo:t - tlo + 1],
                                start=(wsum_n[0] == 1),
                                stop=(wsum_n[0] == NG * TPG * KC),
                                skip_group_check=True)

                def emit_tail2(g, plo=0):
                    attn_g, NB, phi = at_g.pop((g, plo))
                    b0 = g * G + plo
                    # ---- normalizer + store for this chunk ----
                    ps_se = pst_pool.tile([1, 2 * NB], F32, tag="pt",
                                          name=f"ps_se{g}_{plo}")
                    nc.tensor.matmul(ps_se, lhsT=ones_bf[:, 0:1],
                                     rhs=attn_g[:], start=True, stop=True)
                    sums = work.tile([1, NB], F32, tag="se1",
                                     name=f"se{g}_{plo}")
                    nc.vector.reduce_sum(
                        out=sums[:], in_=ps_se.rearrange(
                            "p (b h) -> p b h", h=2), axis=AX.X)
                    rse = work.tile([1, NB], F32, tag="rse",
                                    name=f"rse{g}_{plo}")
                    nc.vector.reciprocal(out=rse[:], in_=sums[:])
                    # broadcast rse over partitions via stride-0 ones matmul
                    ps_rse = pst_pool.tile([128, NB], F32, tag="pt",
                                           name=f"ps_rse{g}_{plo}")
                    nc.tensor.matmul(
                        ps_rse, lhsT=one_f32.to_broadcast([1, 128]),
                        rhs=rse[:], start=True, stop=True)
                    rse_bc = work.tile([128, NB], F32, tag="rsebc",
                                       name=f"rsebc{g}_{plo}")
                    nc.vector.tensor_copy(out=rse_bc[:], in_=ps_rse[:])
                    # normalize + store this chunk's out.T slice
                    oslice = work.tile([128, KC, NB], F32, tag="osl",
                                       name=f"osl{g}_{plo}")
                    nc.vector.tensor_tensor(
                        oslice[:], otp[:, :, b0:b0 + NB],
                        rse_bc[:, None, :].to_broadcast([128, KC, NB]),
                        ALU.mult)
                    nc.sync.dma_start(out=outT[:, :, b0:b0 + NB],
                                      in_=oslice[:])

                # staged emission so the PE queue never head-of-line
                # blocks: front(g) | epilogue+wsum(g-d1) | norm+store(g-d2)
                _d1 = _envint("KB_DEFER", 1)
                _d2 = _d1 + _envint("KB_DEFER2", 0)
                for g in range(NG):
                    emit_front(g)
                    if g >= _d1:
                        emit_tail1(g - _d1)
                    if g >= _d2:
                        emit_tail2(g - _d2)
                for g in range(NG - _d1, NG):
                    emit_tail1(g)
                for g in range(NG - _d2, NG):
                    emit_tail2(g)

    nc.compile()
    return nc


_NC = None


def _get_nc():
    global _NC
    if _NC is None:
        _NC = build_nc()
    return _NC


def _diffuse_fp8(x):
    """Error-diffusion fp8 quantization along the hint-row axis (per
    batch): q_n = fp8(x_n + carry), carry += x_n - q_n."""
    fp8 = ml_dtypes.float8_e4m3
    q = np.empty(x.shape, fp8)
    carry = np.zeros((x.shape[0], x.shape[2]), np.float32)
    for n in range(x.shape[1]):
        v = x[:, n, :] + carry
        qn = v.astype(fp8)
        carry = v - qn.astype(np.float32)
        q[:, n, :] = qn
    return q


def _prep_core_inputs(state_emb, hints_emb, Ws, bs, Wh, bh, core,
                      hints_d8):
    bf16 = ml_dtypes.bfloat16
    s = slice(core * BL, (core + 1) * BL)
    hf = np.ascontiguousarray(hints_emb[s]).reshape(BL * N, HD)
    hf8 = hf.astype(ml_dtypes.float8_e4m3)
    hf8d = hints_d8[s].reshape(BL * N, HD)
    # natural: (g, p, t, f) with row = g*2048 + t*128 + p
    xnat8 = np.ascontiguousarray(
        hf8d.reshape(NG, TPG, 128, 512).transpose(0, 2, 1, 3))
    # transposed: (g, p, t, k, r) with row = g*2048 + t*128 + r, h = k*128+p
    xtd = np.ascontiguousarray(
        hf8.reshape(NG, TPG, 128, KH, 128).transpose(0, 4, 1, 3, 2))
    # Ws.T arranged [s_in_chunk, s_chunk, a]
    wst = Ws.T.reshape(SD // 128, 128, AD).transpose(1, 0, 2)
    wh2 = Wh.reshape(2, 128, HD).transpose(1, 0, 2)
    # S_WH*Wh.T arranged [h_in_chunk, h_chunk, a], fp8 (proj moving operand)
    wht8 = np.ascontiguousarray(
        (Wh.T.reshape(KH, 128, AD).transpose(1, 0, 2) * S_WH)
    ).astype(ml_dtypes.float8_e4m3)
    bh2 = bh.reshape(2, 128).T
    st = np.asarray(state_emb[s])
    # state.T arranged [s_in_chunk, s_chunk, b]
    statet = st.T.reshape(SD // 128, 128, BL).transpose(1, 0, 2)
    wpack = np.concatenate([
        statet.reshape(128, -1), wst.reshape(128, -1),
        wh2.reshape(128, -1),
        bh2.reshape(128, -1),
    ], axis=1).astype(bf16)
    wpack = np.ascontiguousarray(wpack)
    bsbn = np.ascontiguousarray(
        np.broadcast_to(bs, (BL, AD))).astype(np.float32)
    return {
        "wpack": wpack,
        "wht8": wht8,
        "xnat8": xnat8,
        "xt": xtd,
        "bsb": bsbn,
    }


def kernel(state_emb, hints_emb, Ws, bs, Wh, bh):
    state_emb = np.asarray(state_emb, dtype=np.float32)
    hints_emb = np.asarray(hints_emb, dtype=np.float32)
    Ws = np.asarray(Ws, dtype=np.float32)
    bs = np.asarray(bs, dtype=np.float32)
    Wh = np.asarray(Wh, dtype=np.float32)
    bh = np.asarray(bh, dtype=np.float32)

    nc = _get_nc()
    hints_d8 = _diffuse_fp8(hints_emb)
    in_maps = [
        _prep_core_inputs(state_emb, hints_emb, Ws, bs, Wh, bh, c,
                          hints_d8)
        for c in range(NCORES)
    ]
    res = run_bass_kernel_spmd(nc, in_maps, core_ids=list(range(NCORES)))
    outs = []
    for c in range(NCORES):
        oT = np.asarray(res.results[c]["outT"])     # [128, KC, BL]
        outs.append(np.ascontiguousarray(
            oT.transpose(2, 1, 0).reshape(BL, HD)))
    return np.concatenate(outs, axis=0)
